# revision 39
# baseline (speedup 1.0000x reference)
"""Trainium2 Bass kernel for a dense transformer block (pre-LN, MHA + MLP).

Full inputs in, full outputs out. Sharding: 8 cores = (batch, seq-half).
Each core computes K/V over its batch element's full 1024 tokens and
Q/attention/MLP over its own 512 tokens (host permutes tokens so the core's
own half is always rows 0..511 — softmax over keys is permutation invariant).
No collectives needed.

Precision strategy:
  - Attention-side GEMMs (K, V, Q, attn@V, proj) run as fp8e4m3 DoubleRow
    matmuls (two 128-deep contraction slabs per instruction).  Softmax
    normalization + value averaging absorb the fp8 quantization noise.
    Weights are prescaled x16 on the host to sit in e4m3's normal range;
    the 1/16 (or 1/256) correction folds into the psum drain.
  - The MLP (fc1/fc2) stays bf16: fp8 there would blow the 2e-2 error gate.
  - Scores (64-deep contraction) stay bf16; DoubleRow needs 128-pairs.

V is computed token-major directly (stationary = hT chunk, moving = vw
slab) so no PE transposes are needed to build the [k-token, head-dim]
V layout for attn@V.

LayerNorm affine params are folded into the following matmul weights
on the host: (xhat*g + b) @ W == xhat @ (diag(g) W) + b @ W.

Scheduling notes:
  - One dma_start lands on one DMA ring (~22 GB/s), so every transfer
    >=256KB is split into ~128KB chunks across rings.
  - PE work: matmuls + LN transposes.  DVE: LN stats + psum drains.
    Act: exp/gelu/h-write.  Transpose copies alternate DVE/Act.
  - x tiles + fp8 weight slabs live in the scoped `xw` pool, freed after
    proj so the FC pools fit in SBUF.
"""

import sys

sys.path.insert(0, "/opt/trn_rl_repo")

import numpy as np

import concourse.bass as bass
import concourse.bacc as bacc
import concourse.mybir as mybir
import concourse.tile as tile
from concourse.bass_utils import run_bass_kernel_spmd
from concourse.masks import make_identity

P = 128
C = 1024
HEADS = 16
DH = 64
HID = 4096
NTOK = 1024  # tokens per batch element (kv length)
NOWN = 512  # tokens owned by this core (q length)
SCALE = DH ** -0.5
EPS = 1e-5
WS = 16.0  # fp8 weight prescale
RWS = 1.0 / WS

F32 = mybir.dt.float32
BF16 = mybir.dt.bfloat16
FP8 = mybir.dt.float8e4
AF = mybir.ActivationFunctionType
OP = mybir.AluOpType
DR = mybir.MatmulPerfMode.DoubleRow

CT = C // P  # 8 column tiles of the model dim
TT = NTOK // P  # 8 token tiles (kv)
QT = NOWN // P  # 4 token tiles (own)
HT = HID // P  # 32 hidden tiles
HW8 = 80  # per-head column stride in the fused vh tile (64 dims +
# ones col + pad).  The DoubleRow pair stride is TT*HW8... no: the pair
# dim is the k-tile dim with stride HEADS*HW8 = 1280 bytes, a multiple
# of 128 as dual-fp8 ldweights requires.


def build_program():
    nc = bacc.Bacc("TRN2", target_bir_lowering=False)

    io = {}
    io["x"] = nc.dram_tensor("x", (NTOK, C), F32, kind="ExternalInput")
    io["qw"] = nc.dram_tensor("qw", (C, C), FP8, kind="ExternalInput")
    io["kw"] = nc.dram_tensor("kw", (C, C), FP8, kind="ExternalInput")
    io["vw"] = nc.dram_tensor("vw", (C, C), FP8, kind="ExternalInput")
    io["pw"] = nc.dram_tensor("pw", (C, C), FP8, kind="ExternalInput")
    io["f1w"] = nc.dram_tensor("f1w", (C, HID), BF16, kind="ExternalInput")
    io["f2w"] = nc.dram_tensor("f2w", (HID, C), BF16, kind="ExternalInput")
    # biases pre-transposed on host into [128, n] per-partition layout
    io["qbt"] = nc.dram_tensor("qbt", (P, CT), F32, kind="ExternalInput")
    io["kbt"] = nc.dram_tensor("kbt", (P, CT), F32, kind="ExternalInput")
    io["f1bt"] = nc.dram_tensor("f1bt", (P, HT), F32, kind="ExternalInput")
    io["vb"] = nc.dram_tensor("vb", (C,), F32, kind="ExternalInput")
    io["pb"] = nc.dram_tensor("pb", (C,), F32, kind="ExternalInput")
    io["f2b"] = nc.dram_tensor("f2b", (C,), F32, kind="ExternalInput")
    io["out"] = nc.dram_tensor("out", (NOWN, C), F32, kind="ExternalOutput")

    with tile.TileContext(nc) as tc:
        _emit(nc, tc, io)
    nc.compile()
    return nc


def _emit(nc, tc, io):
    x_d, out_d = io["x"], io["out"]

    with (
        tc.tile_pool(name="consts", bufs=1) as consts,
        tc.tile_pool(name="persist", bufs=1) as persist,
    ):

        def copy_alt(i, out, in_):
            nc.any.tensor_copy(out=out, in_=in_)

        # ---------- constants ----------
        ident = consts.tile([P, P], BF16)
        with tc.tile_pool(name="ident_tmp", bufs=1) as ident_tmp:
            ident_f32 = ident_tmp.tile([P, P], F32)
            make_identity(nc, ident_f32)
            nc.vector.tensor_copy(out=ident, in_=ident_f32)
        eps_tile = consts.tile([P, 1], F32)
        nc.vector.memset(eps_tile, EPS)
        eps256 = consts.tile([P, 1], F32)
        nc.vector.memset(eps256, EPS * (WS * WS) ** 2)

        qbT = consts.tile([P, CT], F32)
        nc.sync.dma_start(qbT, io["qbt"][:, :])
        kbT = consts.tile([P, CT], F32)
        nc.sync.dma_start(kbT, io["kbt"][:, :])
        f1bT = consts.tile([P, HT], F32)
        nc.sync.dma_start(f1bT, io["f1bt"][:, :])

        def bcast_const(src_d, n):
            # split across 4 DMA rings; a single 512KB broadcast DMA
            # would occupy one ring for ~20us
            t = consts.tile([P, n], F32)
            for q in range(4):
                nq = n // 4
                src = bass.AP(
                    tensor=src_d, offset=q * nq, ap=[[0, P], [1, nq]]
                )
                nc.sync.dma_start(t[:, q * nq : (q + 1) * nq], src)
            return t

        # x2 split into (tq, ns) halves so LN2 stats can start after the
        # first proj drain of each token tile (deps are tile-granular)
        x2 = {
            (t, ns): persist.tile(
                [P, NOWN], F32, tag=f"x2_{t}_{ns}", name=f"x2_{t}_{ns}"
            )
            for t in range(QT)
            for ns in range(2)
        }

        def layernorm_tile(temps, halves, eps_t=None):
            """halves: two [128, 512] fp32 views/tiles of one token tile.
            Returns two normalized bf16 half tiles [128, 512].

            Stats on DVE, h-write on Act (Identity is in every act table).
            """
            stats = temps.tile([P, 2, 6], F32, tag="ln_stats", name="st")
            for sg in range(2):
                nc.vector.bn_stats(out=stats[:, sg, :], in_=halves[sg])
            mv = temps.tile([P, 2], F32, tag="ln_mv", name="mv")
            nc.vector.bn_aggr(out=mv[:], in_=stats[:])
            rstd = temps.tile([P, 1], F32, tag="ln_rstd", name="rstd")
            nc.scalar.activation(
                out=rstd, in_=mv[:, 1:2], func=AF.Sqrt,
                bias=eps_t if eps_t is not None else eps_tile, scale=1.0,
            )
            nc.vector.reciprocal(out=rstd, in_=rstd)
            nmr = temps.tile([P, 1], F32, tag="ln_nmr", name="nmr")
            nc.vector.tensor_tensor(nmr, mv[:, 0:1], rstd, OP.mult)
            nc.vector.tensor_scalar_mul(nmr, nmr, -1.0)
            hs = []
            for sg in range(2):
                h = temps.tile(
                    [P, NOWN], BF16, tag=f"ln_h{sg}", name=f"h{sg}"
                )
                nc.scalar.activation(
                    out=h, in_=halves[sg], func=AF.Identity,
                    bias=nmr, scale=rstd,
                )
                hs.append(h)
            return hs

        # ---------- persistent attention/MLP SBUF ----------
        # hT[(j, t2)]: [128, 2, 512] fp8, c-slabs (2j, 2j+1), token half t2
        hT = {
            (j, t2): persist.tile(
                [P, 2, NOWN], FP8, tag=f"hT{j}_{t2}", name=f"hT{j}_{t2}"
            )
            for j in range(4)
            for t2 in range(2)
        }
        kT = {
            (ft, t2): persist.tile(
                [P, NOWN], BF16, tag=f"kT{ft}_{t2}", name=f"kT{ft}_{t2}"
            )
            for ft in range(CT)
            for t2 in range(2)
        }
        qT = [
            persist.tile([P, NOWN], BF16, tag=f"qT{ft}", name=f"qT{ft}")
            for ft in range(CT)
        ]
        # vh: [k-token-part, k-tile, head*80 + (64 dims | ones col)] fp8
        vh = persist.tile(
            [P, TT, HEADS * HW8], FP8, tag="vh", name="vh"
        )
        # oT pairs for proj DoubleRow: [feat-part, slab-pair, tok] fp8
        oT = [
            persist.tile([P, 2, NOWN], FP8, tag=f"oT{j}", name=f"oT{j}")
            for j in range(4)
        ]
        h2T = [
            persist.tile([P, NOWN], BF16, tag=f"h2T{c}", name=f"h2T{c}")
            for c in range(CT)
        ]
        vh4 = vh.rearrange("p t (h w) -> p t h w", h=HEADS)
        nc.vector.memset(vh4[:, :, :, DH : DH + 1], 1.0)

        import contextlib
        _xwa_stack = contextlib.ExitStack()
        with tc.tile_pool(name="xwb", bufs=1) as xwb:
            xwa = _xwa_stack.enter_context(tc.tile_pool(name="xwa", bufs=1))
            # ---- x tiles first: LN1 is the critical path at startup ----
            xt_all = []
            for t in range(TT):
                pool_t = xwb if t < QT else xwa
                xo = pool_t.tile([P, C], F32, tag=f"xo{t}", name=f"xo{t}")
                for q in range(4):
                    cs = slice(q * 256, (q + 1) * 256)
                    nc.sync.dma_start(xo[:, cs], x_d[t * P : (t + 1) * P, cs])
                xt_all.append(xo)
            x_own = xt_all[:QT]

            # fp8 weight slabs, all resident (4 MB); V first (used first)
            vslab = [
                xwa.tile([P, CT, NOWN], FP8, tag=f"vs{ns}", name=f"vs{ns}")
                for ns in range(2)
            ]
            for ns in range(2):
                for j in range(4):
                    nc.sync.dma_start(
                        vslab[ns][:, 2 * j : 2 * j + 2, :],
                        io["vw"][
                            2 * j * P : (2 * j + 2) * P,
                            ns * NOWN : (ns + 1) * NOWN,
                        ].rearrange("(c p) n -> p c n", p=P),
                    )
            vb16_bc = bcast_const(io["vb"], C)
            nc.vector.tensor_scalar_mul(vb16_bc, vb16_bc, WS)
            # fp8 copies for the rank-1 bias-add matmul trick
            vb8 = xwa.tile([1, C], FP8, tag="vb8", name="vb8")
            nc.vector.tensor_copy(out=vb8, in_=vb16_bc[0:1, :])
            ones8 = xwa.tile([1, P], FP8, tag="ones8", name="ones8")
            nc.vector.memset(ones8, 1.0)

            kslab = [
                xwa.tile([P, CT, P], FP8, tag=f"ks{ft}", name=f"ks{ft}")
                for ft in range(CT)
            ]
            qslab = [
                xwa.tile([P, CT, P], FP8, tag=f"qs{ft}", name=f"qs{ft}")
                for ft in range(CT)
            ]
            for ft in range(CT):
                nc.sync.dma_start(
                    kslab[ft],
                    io["kw"][:, ft * P : (ft + 1) * P].rearrange(
                        "(c p) f -> p c f", p=P
                    ),
                )
                nc.sync.dma_start(
                    qslab[ft],
                    io["qw"][:, ft * P : (ft + 1) * P].rearrange(
                        "(c p) f -> p c f", p=P
                    ),
                )
            pslab = [
                xwb.tile([P, CT, NOWN], FP8, tag=f"pws{ns}", name=f"pws{ns}")
                for ns in range(2)
            ]
            for ns in range(2):
                for j in range(4):
                    nc.sync.dma_start(
                        pslab[ns][:, 2 * j : 2 * j + 2, :],
                        io["pw"][
                            2 * j * P : (2 * j + 2) * P,
                            ns * NOWN : (ns + 1) * NOWN,
                        ].rearrange("(c p) n -> p c n", p=P),
                    )

            pb_bc = bcast_const(io["pb"], C)
            f2b_bc = bcast_const(io["f2b"], C)

            # ============ Phase 1: LN1 -> hT (fp8, paired slabs) =========
            with (
                tc.tile_pool(name="ln1", bufs=3) as ln1,
                tc.tile_pool(name="tr1", bufs=2, space="PSUM") as tr1,
            ):
                for t in range(TT):
                    xt = xt_all[t]
                    hs = layernorm_tile(
                        ln1, [xt[:, 0:NOWN], xt[:, NOWN:C]]
                    )
                    t2, tb = t // QT, t % QT
                    for c in range(CT):
                        ps = tr1.tile([P, P], BF16, tag="tr", name="tr")
                        src_h = hs[c // 4][:, (c % 4) * P : (c % 4 + 1) * P]
                        nc.tensor.transpose(ps, src_h, ident)
                        copy_alt(
                            c,
                            hT[(c // 2, t2)][:, c % 2, tb * P : (tb + 1) * P],
                            ps,
                        )

            # ========= Phase 2+3: V sweep, then per-ft K/Q + attention ===
            with (
                tc.tile_pool(name="st_ps", bufs=2, space="PSUM") as st_ps,
                tc.tile_pool(name="ot_ps", bufs=2, space="PSUM") as ot_ps,
                tc.tile_pool(name="qkv_ps", bufs=2, space="PSUM") as qkv_ps,
                tc.tile_pool(name="vt_sb", bufs=2) as vt_sb,
                tc.tile_pool(name="den_sb", bufs=2) as den_sb,
                tc.tile_pool(name="p_sb", bufs=2) as p_pool,
            ):
                # -- V: token-major; vh slice = (psum + 16*vb)/16 --
                for c in range(TT):
                    j2, tb = c // QT, c % QT
                    for ns in range(2):
                        ps = qkv_ps.tile([P, NOWN], F32, tag="kvps", name="v")
                        for j in range(4):
                            nc.tensor.matmul(
                                ps,
                                lhsT=hT[(j, j2)][:, :, tb * P : (tb + 1) * P],
                                rhs=vslab[ns][:, 2 * j : 2 * j + 2, :],
                                start=(j == 0),
                                stop=False,
                                perf_mode=DR,
                            )
                        # rank-1 bias add: ones(tok) x 16*vb(feat)
                        nc.tensor.matmul(
                            ps,
                            lhsT=ones8,
                            rhs=vb8[:, ns * NOWN : (ns + 1) * NOWN],
                            start=False,
                            stop=True,
                        )
                        out_view = vh4[:, c, ns * 8 : (ns + 1) * 8, :DH]
                        nc.vector.tensor_scalar(
                            out=out_view, in0=ps, scalar1=RWS,
                            scalar2=None, op0=OP.mult,
                        )

                # -- per-ft: K (both halves), Q, then heads 2ft, 2ft+1 --
                for ft in range(CT):
                    for t2 in range(2):
                        ps = qkv_ps.tile([P, NOWN], F32, tag="kvps", name="k")
                        for j in range(4):
                            nc.tensor.matmul(
                                ps,
                                lhsT=kslab[ft][:, 2 * j : 2 * j + 2, :],
                                rhs=hT[(j, t2)],
                                start=(j == 0),
                                stop=(j == 3),
                                perf_mode=DR,
                            )
                        nc.vector.tensor_scalar(
                            out=kT[(ft, t2)],
                            in0=ps,
                            scalar1=RWS,
                            scalar2=kbT[:, ft : ft + 1],
                            op0=OP.mult,
                            op1=OP.add,
                        )
                    ps = qkv_ps.tile([P, NOWN], F32, tag="kvps", name="q")
                    for j in range(4):
                        nc.tensor.matmul(
                            ps,
                            lhsT=qslab[ft][:, 2 * j : 2 * j + 2, :],
                            rhs=hT[(j, 0)],
                            start=(j == 0),
                            stop=(j == 3),
                            perf_mode=DR,
                        )
                    nc.vector.tensor_scalar(
                        out=qT[ft],
                        in0=ps,
                        scalar1=RWS,
                        scalar2=qbT[:, ft : ft + 1],
                        op0=OP.mult,
                        op1=OP.add,
                    )

                    for hh in range(2):
                        h_idx = 2 * ft + hh
                        prow = hh * DH
                        pg = [
                            p_pool.tile(
                                [P, 2, NOWN], FP8, tag=f"p{g}", name="p"
                            )
                            for g in range(4)
                        ]
                        for g in range(4):  # pairs of k-tiles
                            stg = st_ps.tile(
                                [P, 2, NOWN], F32, tag="st", name="st"
                            )
                            for i in range(2):
                                c = 2 * g + i
                                nc.tensor.matmul(
                                    stg[:, i, :],
                                    lhsT=kT[(ft, c // 4)][
                                        prow : prow + DH,
                                        (c % 4) * P : (c % 4 + 1) * P,
                                    ],
                                    rhs=qT[ft][prow : prow + DH, :],
                                    start=True,
                                    stop=True,
                                )
                            nc.scalar.activation(
                                out=pg[g], in_=stg, func=AF.Exp, scale=SCALE
                            )
                        ot = ot_ps.tile([P, NOWN], F32, tag="ot", name="ot")
                        hc0 = h_idx * HW8
                        for j in range(4):
                            nc.tensor.matmul(
                                ot[: DH + 1, :],
                                lhsT=vh[
                                    :, 2 * j : 2 * j + 2, hc0 : hc0 + DH + 1
                                ],
                                rhs=pg[j],
                                start=(j == 0),
                                stop=(j == 3),
                                perf_mode=DR,
                            )
                        # softmax denominator in row DH (vh ones column);
                        # oT = 16 * o / den  (x16 = fp8 range for proj)
                        # rs = 16/den: stage den/16 in SBUF (the custom
                        # DVE reciprocal op cannot read PSUM), then invert
                        dsb = den_sb.tile([1, NOWN], F32, tag="dsb", name="d")
                        nc.vector.tensor_scalar(
                            out=dsb, in0=ot[DH : DH + 1, :], scalar1=RWS,
                            scalar2=None, op0=OP.mult,
                        )
                        rs = den_sb.tile([1, NOWN], F32, tag="rs", name="rs")
                        nc.vector.reciprocal_approx_fast(out=rs, in_=dsb)
                        rsb = den_sb.tile([DH, NOWN], F32, tag="rb", name="rb")
                        nc.gpsimd.partition_broadcast(rsb, rs)
                        nc.vector.tensor_tensor(
                            oT[ft // 2][prow : prow + DH, ft % 2, :],
                            ot[:DH, :],
                            rsb,
                            OP.mult,
                        )

            _xwa_stack.close()  # frees x_other + k/q/v slabs for FC pools

            # 256*(x_own + pb) precomputed so the proj drain is ONE add:
            # x2 is accumulated at 256x scale (psum already carries the
            # 16*16 weight prescales); LN2 is scale-invariant with eps
            # scaled by 256^2, and FC2's residual applies the 1/256.
            xpb = [
                xwb.tile([P, C], F32, tag=f"xpb{t}", name=f"xpb{t}")
                for t in range(QT)
            ]
            for t in range(QT):
                nc.vector.tensor_add(xpb[t], x_own[t], pb_bc)
                nc.vector.tensor_scalar_mul(xpb[t], xpb[t], WS * WS)

            # ==== Phase 4+5: proj + residual -> x2, LN2 -> h2T (per tq) ==
            with (
                tc.tile_pool(name="proj_ps", bufs=4, space="PSUM") as proj_ps,
                tc.tile_pool(name="tr2", bufs=2, space="PSUM") as tr2,
                tc.tile_pool(name="ln2", bufs=2) as ln2,
            ):
                for tq in range(QT):
                    for ns in range(2):
                        nsl = slice(ns * NOWN, (ns + 1) * NOWN)
                        ps = proj_ps.tile([P, NOWN], F32, tag="pps", name="pp")
                        for j2 in range(4):
                            nc.tensor.matmul(
                                ps,
                                lhsT=oT[j2][:, :, tq * P : (tq + 1) * P],
                                rhs=pslab[ns][:, 2 * j2 : 2 * j2 + 2, :],
                                start=(j2 == 0),
                                stop=(j2 == 3),
                                perf_mode=DR,
                            )
                        # x2_256 = psum + 256*(pb + x_own)
                        nc.vector.tensor_add(
                            x2[(tq, ns)], ps, xpb[tq][:, nsl]
                        )
                    hs = layernorm_tile(
                        ln2, [x2[(tq, 0)], x2[(tq, 1)]], eps_t=eps256
                    )
                    for c in range(CT):
                        ps2 = tr2.tile([P, P], BF16, tag="tr", name="tr")
                        src_h = hs[c // 4][:, (c % 4) * P : (c % 4 + 1) * P]
                        nc.tensor.transpose(ps2, src_h, ident)
                        copy_alt(c, h2T[c][:, tq * P : (tq + 1) * P], ps2)

        # ============ Phase 6+7: FC1 + gelu -> actT, FC2 + residual ======
        actT = [
            persist.tile([P, NOWN], BF16, tag=f"actT{hf}", name=f"actT{hf}")
            for hf in range(HT)
        ]
        with (
            tc.tile_pool(name="f1c", bufs=4) as f1c,
            tc.tile_pool(name="f1_ps", bufs=4, space="PSUM") as f1_ps,
            tc.tile_pool(name="f2c", bufs=1) as f2c,
            tc.tile_pool(name="f2_ps", bufs=2, space="PSUM") as f2_ps,
            tc.tile_pool(name="out_sb", bufs=2) as out_pool,
        ):
            # prefetch first fc2 half during fc1
            groups0 = []
            for g in range(8):
                gw = f2c.tile([P, 4, NOWN], BF16, tag=f"g{g}", name=f"fg{g}0")
                nc.sync.dma_start(
                    gw,
                    io["f2w"][g * NOWN : (g + 1) * NOWN, 0:NOWN].rearrange(
                        "(o p) n -> p o n", p=P
                    ),
                )
                groups0.append(gw)

            for hf in range(HT):
                ps = f1_ps.tile([P, NOWN], F32, tag="f1ps", name="f1ps")
                slab = f1c.tile([P, CT, P], BF16, tag="f1w", name="f1slab")
                nc.sync.dma_start(
                    slab,
                    io["f1w"][:, hf * P : (hf + 1) * P].rearrange(
                        "(c p) f -> p c f", p=P
                    ),
                )
                for c in range(CT):
                    nc.tensor.matmul(
                        ps,
                        lhsT=slab[:, c, :],
                        rhs=h2T[c],
                        start=(c == 0),
                        stop=(c == CT - 1),
                    )
                nc.scalar.activation(
                    out=actT[hf],
                    in_=ps,
                    func=AF.Gelu,
                    bias=f1bT[:, hf : hf + 1],
                    scale=1.0,
                )

            for ns in range(2):
                nsl = slice(ns * NOWN, (ns + 1) * NOWN)
                if ns == 0:
                    groups = groups0
                else:
                    groups = []
                    for g in range(8):
                        gw = f2c.tile(
                            [P, 4, NOWN], BF16, tag=f"g{g}", name=f"fg{g}1"
                        )
                        nc.sync.dma_start(
                            gw,
                            io["f2w"][g * NOWN : (g + 1) * NOWN, nsl].rearrange(
                                "(o p) n -> p o n", p=P
                            ),
                        )
                        groups.append(gw)
                for tq in range(QT):
                    ps = f2_ps.tile([P, NOWN], F32, tag="f2ps", name="f2ps")
                    for hc in range(HT):
                        nc.tensor.matmul(
                            ps,
                            lhsT=actT[hc][:, tq * P : (tq + 1) * P],
                            rhs=groups[hc // 4][:, hc % 4, :],
                            start=(hc == 0),
                            stop=(hc == HT - 1),
                        )
                    ot2 = out_pool.tile([P, NOWN], F32, tag="out_t", name="o")
                    nc.vector.tensor_add(ps, ps, f2b_bc[:, nsl])
                    nc.vector.tensor_scalar(
                        out=ot2, in0=x2[(tq, ns)],
                        scalar1=1.0 / (WS * WS), scalar2=None, op0=OP.mult,
                    )
                    nc.vector.tensor_add(ot2, ps, ot2)
                    nc.sync.dma_start(out_d[tq * P : (tq + 1) * P, nsl], ot2)


_PROGRAM = None


def _get_program():
    global _PROGRAM
    if _PROGRAM is None:
        _PROGRAM = build_program()
    return _PROGRAM


def build_in_maps(inputs):
    import ml_dtypes

    E4 = ml_dtypes.float8_e4m3

    x = np.asarray(inputs["x"], np.float32)  # [4, 1024, 1024]
    ln1_g = np.asarray(inputs["ln1_g"], np.float64)
    ln1_b = np.asarray(inputs["ln1_b"], np.float64)
    ln2_g = np.asarray(inputs["ln2_g"], np.float64)
    ln2_b = np.asarray(inputs["ln2_b"], np.float64)
    qkv_w = np.asarray(inputs["qkv_w"], np.float64)
    qkv_b = np.asarray(inputs["qkv_b"], np.float64)
    proj_w = np.asarray(inputs["proj_w"], np.float64)
    proj_b = np.asarray(inputs["proj_b"], np.float32)
    fc1_w = np.asarray(inputs["fc1_w"], np.float64)
    fc1_b = np.asarray(inputs["fc1_b"], np.float64)
    fc2_w = np.asarray(inputs["fc2_w"], np.float64)
    fc2_b = np.asarray(inputs["fc2_b"], np.float32)

    # Fold LN affine into the following matmul:
    #   (xhat*g + b) @ W == xhat @ (diag(g) W) + b @ W
    qkv_w_f = ln1_g[:, None] * qkv_w
    qkv_b_f = (qkv_b + ln1_b @ qkv_w).astype(np.float32)
    f1w_f = ln2_g[:, None] * fc1_w
    f1b_f = (fc1_b + ln2_b @ fc1_w).astype(np.float32)

    qw8 = (qkv_w_f[:, :C] * WS).astype(np.float32).astype(E4)
    kw8 = (qkv_w_f[:, C : 2 * C] * WS).astype(np.float32).astype(E4)
    vw8 = (qkv_w_f[:, 2 * C :] * WS).astype(np.float32).astype(E4)
    pw8 = (proj_w * WS).astype(np.float32).astype(E4)
    f1w16 = f1w_f.astype(ml_dtypes.bfloat16)
    f2w16 = fc2_w.astype(ml_dtypes.bfloat16)

    def tbias(b):  # [n*128] -> [128, n] per-partition layout
        return np.ascontiguousarray(b.reshape(-1, P).T)

    common = dict(
        qw=np.ascontiguousarray(qw8),
        kw=np.ascontiguousarray(kw8),
        vw=np.ascontiguousarray(vw8),
        pw=np.ascontiguousarray(pw8),
        f1w=np.ascontiguousarray(f1w16),
        f2w=np.ascontiguousarray(f2w16),
        qbt=tbias(qkv_b_f[:C]),
        kbt=tbias(qkv_b_f[C : 2 * C]),
        f1bt=tbias(f1b_f),
        vb=np.ascontiguousarray(qkv_b_f[2 * C :]),
        pb=proj_b,
        f2b=fc2_b,
    )
    in_maps = []
    for core in range(8):
        b, half = core // 2, core % 2
        own = x[b, half * NOWN : (half + 1) * NOWN, :]
        other = x[b, (1 - half) * NOWN : (2 - half) * NOWN, :]
        xp = np.ascontiguousarray(np.concatenate([own, other], axis=0))
        in_maps.append({**common, "x": xp})
    return in_maps


def kernel(**inputs):
    in_maps = build_in_maps(inputs)
    nc = _get_program()
    res = run_bass_kernel_spmd(nc, in_maps, core_ids=list(range(8)))
    outs = res.results

    y = np.empty((4, NTOK, C), np.float32)
    for core in range(8):
        b, half = core // 2, core % 2
        y[b, half * NOWN : (half + 1) * NOWN, :] = outs[core]["out"]
    return y


if __name__ == "__main__":
    prog = build_program()
    print("program built OK")
